# revision 18
# baseline (speedup 1.0000x reference)
"""Multi-head attention on 8 Trainium2 NeuronCores (Bass/Tile).

Sharding: batch B=4 x head-groups 2 -> 8 cores. Each core computes full
attention for 1 batch element and 8 of 16 heads, producing a partial
output projection (Wo row-sharded); host sums the two partials per batch.

Device dataflow (per core), everything in "transposed" orientation so the
contraction dim always sits on SBUF partitions. All matmul operands are
bf16 (fp32 PSUM accumulation); fp32r measured 3x slower per column on HW.
  qT/kT/vT [DM=1024, S=2048] bf16 from host (host pre-transposes+casts).
  QT = (Wq^T qT) [512, S] bf16  (DVE adds bq while copying PSUM->SBUF)
  KT likewise; V natural [S, 512] via lhsT=vT s-tiles (bv added with a
  K=1 ones-row matmul), stored per head with an appended ones column:
  Vp [k-tile, head, 65] bf16.
  scores^T[k,q] = (K_h^T tile).T @ Q_h^T, N=1024 (K=64 contraction; even/
  odd heads on partitions 0-63/64-127 -> concurrent PE row-tiles).
  expS = Exp(scores * 0.125) ACT PSUM->SBUF bf16, [128,1024] chunks.
  PV: out'[65, q] += Vp_tile.T @ expS_tile accumulated over 16 k-tiles;
  row 64 is the softmax denominator (ones column trick).
  A^T = out'[0:64] * reciprocal(out'[64]) broadcast -> bf16 [512, S].
  O^T[m, s] = Wo_chunk.T @ A^T chunk accumulated over 4 chunks -> fp32.
Host: out[b] = (O^T_hg0 + O^T_hg1).T + bo.
"""

import sys

sys.path.insert(0, "/opt/trn_rl_repo")

import ml_dtypes
import numpy as np

import concourse.bacc as bacc
import concourse.mybir as mybir
from concourse import tile
from concourse.bass_utils import run_bass_kernel_spmd

F32 = mybir.dt.float32
BF16 = mybir.dt.bfloat16
AF = mybir.ActivationFunctionType
NP_BF16 = ml_dtypes.bfloat16

H, DK, DV, DM = 16, 64, 64, 1024
B, S = 4, 2048
HL = H // 2          # heads per core
NB = HL * DK         # 512: per-core projection width
NDM = DM // 128      # 8 contraction chunks
NT = NB // 128       # 4 row-tiles of QT/KT/AT
NKT = S // 128       # 16 k-tiles
NQB = S // 1024      # 2 q-blocks of 1024
SCALE = 1.0 / 8.0    # 1/sqrt(DK)

_CACHED_NC = None

import os

DEBUG_DUMP = bool(os.environ.get("KERNEL_DEBUG_DUMP"))


def _build():
    nc = bacc.Bacc("TRN2", debug=False)

    qT = nc.dram_tensor("qT", [DM, S], BF16, kind="ExternalInput")
    kT = nc.dram_tensor("kT", [DM, S], BF16, kind="ExternalInput")
    vT = nc.dram_tensor("vT", [DM, S], BF16, kind="ExternalInput")
    wq = nc.dram_tensor("wq", [DM, NB], BF16, kind="ExternalInput")
    wk = nc.dram_tensor("wk", [DM, NB], BF16, kind="ExternalInput")
    wv = nc.dram_tensor("wv", [DM, NB], BF16, kind="ExternalInput")
    wo = nc.dram_tensor("wo", [NB, DM], BF16, kind="ExternalInput")
    bq = nc.dram_tensor("bq", [NB], F32, kind="ExternalInput")
    bk = nc.dram_tensor("bk", [NB], F32, kind="ExternalInput")
    bv = nc.dram_tensor("bv", [NB], BF16, kind="ExternalInput")
    ones = nc.dram_tensor("ones", [1, 128], BF16, kind="ExternalInput")
    outT = nc.dram_tensor("outT", [DM, S], F32, kind="ExternalOutput")
    if DEBUG_DUMP:
        qt_dbg = nc.dram_tensor("qt_dbg", [128, NT, S], BF16, kind="ExternalOutput")
        kt_dbg = nc.dram_tensor("kt_dbg", [128, NT, S], BF16, kind="ExternalOutput")
        vp_dbg = nc.dram_tensor(
            "vp_dbg", [128, NKT, HL, DV + 1], BF16, kind="ExternalOutput"
        )
        at_dbg = nc.dram_tensor("at_dbg", [128, NT, S], BF16, kind="ExternalOutput")
        ex_dbg = nc.dram_tensor("ex_dbg", [128, 1024], BF16, kind="ExternalOutput")
        pv_dbg = nc.dram_tensor("pv_dbg", [128, 1024], F32, kind="ExternalOutput")

    with tile.TileContext(nc) as tc:
        with tc.tile_pool(name="persist", bufs=1) as persist:
            QT = persist.tile([128, NT, S], BF16)
            KT = persist.tile([128, NT, S], BF16)
            Vp = persist.tile([128, NKT, HL, DV + 1], BF16)
            wo_sb = persist.tile([128, NT, DM], BF16)
            bq_sb = persist.tile([128, NT], F32)
            bk_sb = persist.tile([128, NT], F32)
            bv_sb = persist.tile([1, NB], BF16)
            ones_sb = persist.tile([1, 128], BF16)

            nc.sync.dma_start(wo_sb[:], wo.rearrange("(c p) m -> p c m", p=128))
            nc.sync.dma_start(bq_sb[:], bq.rearrange("(t p) -> p t", p=128))
            nc.sync.dma_start(bk_sb[:], bk.rearrange("(t p) -> p t", p=128))
            nc.sync.dma_start(bv_sb[:], bv.rearrange("(o n) -> o n", o=1))
            nc.sync.dma_start(ones_sb[:], ones[:])
            nc.vector.memset(Vp[:, :, :, DV : DV + 1], 1.0)

            # ---- Stage 1: projections ----
            with (
                tc.tile_pool(name="acts", bufs=3) as acts_pool,
                tc.tile_pool(name="wgt", bufs=2) as wgt_pool,
                tc.tile_pool(name="ps_proj", bufs=2, space="PSUM") as ps_proj,
            ):
                for src_act, src_w, bias_sb, dstT in (
                    (qT, wq, bq_sb, QT),
                    (kT, wk, bk_sb, KT),
                ):
                    wt = wgt_pool.tile([128, NDM, NB], BF16, tag="w")
                    nc.sync.dma_start(
                        wt[:], src_w.rearrange("(c p) n -> p c n", p=128)
                    )
                    for sq in range(NQB):
                        act = acts_pool.tile([128, NDM, 1024], BF16, tag="a")
                        nc.sync.dma_start(
                            act[:],
                            src_act[:, sq * 1024 : (sq + 1) * 1024].rearrange(
                                "(c p) s -> p c s", p=128
                            ),
                        )
                        for t in range(NT):
                            ps = ps_proj.tile([128, 1024], F32, tag="ps")
                            for half in range(2):
                                hs = slice(half * 512, half * 512 + 512)
                                for c in range(NDM):
                                    nc.tensor.matmul(
                                        ps[:, hs],
                                        wt[:, c, t * 128 : (t + 1) * 128],
                                        act[:, c, hs],
                                        start=(c == 0),
                                        stop=(c == NDM - 1),
                                    )
                            nc.vector.tensor_scalar_add(
                                dstT[:, t, sq * 1024 : (sq + 1) * 1024],
                                ps[:],
                                bias_sb[:, t : t + 1],
                            )

                # V projection: natural orientation, bias via ones-row matmul
                wt = wgt_pool.tile([128, NDM, NB], BF16, tag="w")
                nc.sync.dma_start(wt[:], wv.rearrange("(c p) n -> p c n", p=128))
                for sq in range(NQB):
                    act = acts_pool.tile([128, NDM, 1024], BF16, tag="a")
                    nc.sync.dma_start(
                        act[:],
                        vT[:, sq * 1024 : (sq + 1) * 1024].rearrange(
                            "(c p) s -> p c s", p=128
                        ),
                    )
                    for sti in range(8):
                        st = sq * 8 + sti
                        ps = ps_proj.tile([128, 512], F32, tag="ps")
                        for c in range(NDM):
                            nc.tensor.matmul(
                                ps[:],
                                act[:, c, sti * 128 : (sti + 1) * 128],
                                wt[:, c, :],
                                start=(c == 0),
                                stop=False,
                            )
                        nc.tensor.matmul(
                            ps[:],
                            ones_sb[0:1, :],
                            bv_sb[0:1, :],
                            start=False,
                            stop=True,
                        )
                        nc.vector.tensor_copy(
                            Vp[:, st, :, 0:DV],
                            ps[:].rearrange("p (h d) -> p h d", h=HL),
                        )

            if DEBUG_DUMP:
                nc.sync.dma_start(qt_dbg[:], QT[:])
                nc.sync.dma_start(kt_dbg[:], KT[:])
                nc.sync.dma_start(vp_dbg[:], Vp[:])

            # ---- Stage 2: attention ----
            with tc.tile_pool(name="att", bufs=1) as att_pool:
                AT = att_pool.tile([128, NT, S], BF16)
                with (
                    tc.tile_pool(name="expS", bufs=5) as exp_pool,
                    tc.tile_pool(name="rec", bufs=2) as rec_pool,
                    tc.tile_pool(name="ps_sc", bufs=1, space="PSUM") as ps_sc,
                    tc.tile_pool(name="ps_pv", bufs=2, space="PSUM") as ps_pv,
                ):
                    for hp in range(HL // 2):  # head pairs: even head on
                        t = hp                 # partitions 0-63, odd on 64-127
                        for qb in range(NQB):
                            qsl = slice(qb * 1024, (qb + 1) * 1024)
                            # Per k-tile: both heads' scores into one 4-bank
                            # PSUM tile, a single [128, 2048]-free exp, and the
                            # previous k-tile's PV matmuls emitted in between
                            # so the PE keeps ~85% duty (HAM stays warm) while
                            # ACT paces the loop.
                            pv_list = [
                                ps_pv.tile([128, 1024], F32, tag="pv", name=f"pv{i}")
                                for i in range(2)
                            ]

                            def emit_pv(kt, ex):
                                for sub in range(2):
                                    for half in range(2):
                                        hs = slice(half * 512, half * 512 + 512)
                                        nc.tensor.matmul(
                                            pv_list[sub][0 : DV + 1, hs],
                                            Vp[:, kt, hp * 2 + sub, :],
                                            ex[:, sub, hs],
                                            start=(kt == 0),
                                            stop=(kt == NKT - 1),
                                        )

                            ex_tiles = []
                            for kt in range(NKT):
                                psl0 = slice(0, 64)
                                scp = ps_sc.tile([128, 2, 1024], F32, tag="sc")
                                for sub in range(2):
                                    psl = slice(sub * 64, sub * 64 + 64)
                                    for half in range(2):
                                        hs = slice(
                                            qb * 1024 + half * 512,
                                            qb * 1024 + half * 512 + 512,
                                        )
                                        nc.tensor.matmul(
                                            scp[:, sub, half * 512 : half * 512 + 512],
                                            KT[psl, t, kt * 128 : (kt + 1) * 128],
                                            QT[psl, t, hs],
                                            start=True,
                                            stop=True,
                                        )
                                ex = exp_pool.tile([128, 2, 1024], BF16, tag="e")
                                nc.scalar.activation(
                                    ex[:], scp[:], AF.Exp, scale=SCALE
                                )
                                if DEBUG_DUMP and hp == 0 and qb == 0 and kt == 0:
                                    nc.sync.dma_start(ex_dbg[:], ex[:, 0, :])
                                ex_tiles.append(ex)
                                if kt > 0:
                                    emit_pv(kt - 1, ex_tiles[kt - 1])
                            emit_pv(NKT - 1, ex_tiles[NKT - 1])
                            if DEBUG_DUMP and hp == 0 and qb == 0:
                                pv_stg = rec_pool.tile(
                                    [128, 1024], F32, tag="pvstg"
                                )
                                nc.vector.tensor_copy(pv_stg[:], pv_list[0][:])
                                nc.sync.dma_start(pv_dbg[:], pv_stg[:])
                            for sub in range(2):
                                psl = slice(sub * 64, sub * 64 + 64)
                                pvp = pv_list[sub]
                                rec = rec_pool.tile([1, 1024], F32, tag="r")
                                recb = rec_pool.tile([64, 1024], F32, tag="rb")
                                dcp = rec_pool.tile([1, 1024], F32, tag="d")
                                # custom-DVE ucode mishandles base_partition=64
                                # PSUM reads in-kernel; stage through partition 0
                                nc.vector.tensor_copy(dcp[:], pvp[DV : DV + 1, :])
                                nc.vector.reciprocal_approx_fast(rec[:], dcp[:])
                                nc.gpsimd.partition_broadcast(recb[:], rec[:])
                                nc.vector.tensor_mul(
                                    AT[psl, t, qsl], pvp[0:DV, :], recb[:]
                                )

                if DEBUG_DUMP:
                    nc.sync.dma_start(at_dbg[:], AT[:])

                # ---- Stage 3: output projection ----
                with (
                    tc.tile_pool(name="ostage", bufs=3) as ostage,
                    tc.tile_pool(name="ps_o", bufs=2, space="PSUM") as ps_o,
                ):
                    for m in range(NDM):
                        for sbk in range(NQB):
                            ps = ps_o.tile([128, 1024], F32, tag="po")
                            for half in range(2):
                                hs = slice(
                                    sbk * 1024 + half * 512,
                                    sbk * 1024 + half * 512 + 512,
                                )
                                for cc in range(NT):
                                    nc.tensor.matmul(
                                        ps[:, half * 512 : half * 512 + 512],
                                        wo_sb[:, cc, m * 128 : (m + 1) * 128],
                                        AT[:, cc, hs],
                                        start=(cc == 0),
                                        stop=(cc == NT - 1),
                                    )
                            ot = ostage.tile([128, 1024], F32, tag="o")
                            nc.vector.tensor_copy(ot[:], ps[:])
                            nc.sync.dma_start(
                                outT[
                                    m * 128 : (m + 1) * 128,
                                    sbk * 1024 : (sbk + 1) * 1024,
                                ],
                                ot[:],
                            )

    nc.compile()
    return nc


def get_nc():
    global _CACHED_NC
    if _CACHED_NC is None:
        _CACHED_NC = _build()
    return _CACHED_NC


def _bf(x):
    return np.ascontiguousarray(np.asarray(x, np.float32)).astype(NP_BF16)


def make_in_maps(queries, keys, values, Wq, bq, Wk, bk, Wv, bv, Wo, bo):
    queries = np.asarray(queries, np.float32)
    keys = np.asarray(keys, np.float32)
    values = np.asarray(values, np.float32)
    Wq = np.asarray(Wq, np.float32)
    Wk = np.asarray(Wk, np.float32)
    Wv = np.asarray(Wv, np.float32)
    Wo = np.asarray(Wo, np.float32)
    bq = np.asarray(bq, np.float32)
    bk = np.asarray(bk, np.float32)
    bv = np.asarray(bv, np.float32)
    ones = np.ones((1, 128), NP_BF16)
    in_maps = []
    for core in range(8):
        b, hg = divmod(core, 2)
        sl = slice(hg * NB, (hg + 1) * NB)
        in_maps.append(
            {
                "qT": _bf(queries[b].T),
                "kT": _bf(keys[b].T),
                "vT": _bf(values[b].T),
                "wq": _bf(Wq[:, sl]),
                "wk": _bf(Wk[:, sl]),
                "wv": _bf(Wv[:, sl]),
                "wo": _bf(Wo[sl, :]),
                "bq": np.ascontiguousarray(bq[sl]),
                "bk": np.ascontiguousarray(bk[sl]),
                "bv": _bf(bv[sl]),
                "ones": ones,
            }
        )
    return in_maps


def assemble(results, bo):
    bo = np.asarray(bo, np.float32)
    out = np.empty((B, S, DM), np.float32)
    for b in range(B):
        acc = results[2 * b]["outT"] + results[2 * b + 1]["outT"]
        out[b] = acc.T + bo
    return out


def run(trace=False, **inputs):
    if trace:
        # NTFF profiling shim: this image's antenv lacks axon_hooks.
        import types

        try:
            from antenv import axon_hooks  # noqa: F401
        except ImportError:
            from trn_agent_boot.trn_boot import _ntff_profile_via_ctypes

            mod = types.ModuleType("antenv.axon_hooks")
            _hook = _ntff_profile_via_ctypes("/opt/axon/libaxon_pjrt.so")
            mod.get_axon_ntff_profile_hook = lambda: _hook
            sys.modules["antenv.axon_hooks"] = mod
    nc = get_nc()
    bo = inputs["bo"]
    in_maps = make_in_maps(**inputs)
    res = run_bass_kernel_spmd(nc, in_maps, list(range(8)), trace=trace)
    return assemble(res.results, bo), res


def kernel(**inputs):
    out, _ = run(trace=False, **inputs)
    return out


# revision 21
# speedup vs baseline: 1.2458x; 1.2458x over previous
"""Multi-head attention on 8 Trainium2 NeuronCores (Bass/Tile).

Sharding: batch B=4 x head-groups 2 -> 8 cores. Each core computes full
attention for 1 batch element and 8 of 16 heads, producing a partial
output projection (Wo row-sharded); host sums the two partials per batch.

Device dataflow (per core), everything in "transposed" orientation so the
contraction dim always sits on SBUF partitions. All matmul operands are
bf16 (fp32 PSUM accumulation); fp32r measured 3x slower per column on HW.
  qT/kT/vT [DM=1024, S=2048] bf16 from host (host pre-transposes+casts).
  QT = (Wq^T qT) [512, S] bf16  (DVE adds bq while copying PSUM->SBUF)
  KT likewise; V natural [S, 512] via lhsT=vT s-tiles (bv added with a
  K=1 ones-row matmul), stored per head with an appended ones column:
  Vp [k-tile, head, 65] bf16.
  scores^T[k,q] = (K_h^T tile).T @ Q_h^T, N=1024 (K=64 contraction; even/
  odd heads on partitions 0-63/64-127 -> concurrent PE row-tiles).
  expS = Exp(scores * 0.125) ACT PSUM->SBUF bf16, [128,1024] chunks.
  PV: out'[65, q] += Vp_tile.T @ expS_tile accumulated over 16 k-tiles;
  row 64 is the softmax denominator (ones column trick).
  A^T = out'[0:64] * reciprocal(out'[64]) broadcast -> bf16 [512, S].
  O^T[m, s] = Wo_chunk.T @ A^T chunk accumulated over 4 chunks -> fp32.
Host: out[b] = (O^T_hg0 + O^T_hg1).T + bo.
"""

import sys

sys.path.insert(0, "/opt/trn_rl_repo")

import ml_dtypes
import numpy as np

import concourse.bacc as bacc
import concourse.mybir as mybir
from concourse import tile
from concourse.bass_utils import run_bass_kernel_spmd

F32 = mybir.dt.float32
BF16 = mybir.dt.bfloat16
AF = mybir.ActivationFunctionType
NP_BF16 = ml_dtypes.bfloat16

H, DK, DV, DM = 16, 64, 64, 1024
B, S = 4, 2048
HL = H // 2          # heads per core
NB = HL * DK         # 512: per-core projection width
NDM = DM // 128      # 8 contraction chunks
NT = NB // 128       # 4 row-tiles of QT/KT/AT
NKT = S // 128       # 16 k-tiles
NQB = S // 1024      # 2 q-blocks of 1024
SCALE = 1.0 / 8.0    # 1/sqrt(DK)

_CACHED_NC = None

import os

DEBUG_DUMP = bool(os.environ.get("KERNEL_DEBUG_DUMP"))


def _build():
    nc = bacc.Bacc("TRN2", debug=False)

    qT = nc.dram_tensor("qT", [DM, S], BF16, kind="ExternalInput")
    kT = nc.dram_tensor("kT", [DM, S], BF16, kind="ExternalInput")
    vT = nc.dram_tensor("vT", [DM, S], BF16, kind="ExternalInput")
    wq = nc.dram_tensor("wq", [DM, NB], BF16, kind="ExternalInput")
    wk = nc.dram_tensor("wk", [DM, NB], BF16, kind="ExternalInput")
    wv = nc.dram_tensor("wv", [DM, NB], BF16, kind="ExternalInput")
    wo = nc.dram_tensor("wo", [NB, DM], BF16, kind="ExternalInput")
    bq = nc.dram_tensor("bq", [NB], F32, kind="ExternalInput")
    bk = nc.dram_tensor("bk", [NB], F32, kind="ExternalInput")
    bv = nc.dram_tensor("bv", [NB], BF16, kind="ExternalInput")
    ones = nc.dram_tensor("ones", [1, 128], BF16, kind="ExternalInput")
    outT = nc.dram_tensor("outT", [DM, S], F32, kind="ExternalOutput")
    if DEBUG_DUMP:
        qt_dbg = nc.dram_tensor("qt_dbg", [128, NT, S], BF16, kind="ExternalOutput")
        kt_dbg = nc.dram_tensor("kt_dbg", [128, NT, S], BF16, kind="ExternalOutput")
        vp_dbg = nc.dram_tensor(
            "vp_dbg", [128, NKT, HL, DV + 1], BF16, kind="ExternalOutput"
        )
        at_dbg = nc.dram_tensor("at_dbg", [128, NT, S], BF16, kind="ExternalOutput")
        ex_dbg = nc.dram_tensor("ex_dbg", [128, 512], BF16, kind="ExternalOutput")
        pv_dbg = nc.dram_tensor("pv_dbg", [128, 512], F32, kind="ExternalOutput")

    with tile.TileContext(nc) as tc:
        with tc.tile_pool(name="persist", bufs=1) as persist:
            QT = persist.tile([128, NT, S], BF16)
            KT = persist.tile([128, NT, S], BF16)
            Vp = persist.tile([128, NKT, HL, DV + 1], BF16)
            wo_sb = persist.tile([128, NT, DM], BF16)
            bq_sb = persist.tile([128, NT], F32)
            bk_sb = persist.tile([128, NT], F32)
            bv_sb = persist.tile([1, NB], BF16)
            ones_sb = persist.tile([1, 128], BF16)

            nc.sync.dma_start(wo_sb[:], wo.rearrange("(c p) m -> p c m", p=128))
            nc.sync.dma_start(bq_sb[:], bq.rearrange("(t p) -> p t", p=128))
            nc.sync.dma_start(bk_sb[:], bk.rearrange("(t p) -> p t", p=128))
            nc.sync.dma_start(bv_sb[:], bv.rearrange("(o n) -> o n", o=1))
            nc.sync.dma_start(ones_sb[:], ones[:])
            nc.vector.memset(Vp[:, :, :, DV : DV + 1], 1.0)

            # ---- Stage 1: V projection + Q/K projections for t=0 ----
            # Q/K projections for t=1..3 are interleaved into the attention
            # loop below (one matmul per k-tile tick) so they fill the PE
            # while ACT paces the softmax and the HAM clock stays warm.
            acts_pool = tc.alloc_tile_pool(name="acts", bufs=2)
            wgt_pool = tc.alloc_tile_pool(name="wgt", bufs=1)

            wts = {}
            for key, src_w in (("q", wq), ("k", wk), ("v", wv)):
                wt = wgt_pool.tile([128, NDM, NB], BF16, tag=f"w{key}", name=f"w{key}")
                nc.sync.dma_start(wt[:], src_w.rearrange("(c p) n -> p c n", p=128))
                wts[key] = wt
            srcs = {"q": qT, "k": kT, "v": vT}
            biases = {"q": bq_sb, "k": bk_sb}
            dsts = {"q": QT, "k": KT}

            def load_act(key, sq):
                act = acts_pool.tile([128, NDM, 1024], BF16, tag="a", name=f"a{key}{sq}")
                nc.sync.dma_start(
                    act[:],
                    srcs[key][:, sq * 1024 : (sq + 1) * 1024].rearrange(
                        "(c p) s -> p c s", p=128
                    ),
                )
                return act

            with tc.tile_pool(name="ps_s1", bufs=4, space="PSUM") as ps_s1:
                # Q/K projections for t=0
                for key in ("q", "k"):
                    for sq in range(NQB):
                        act = load_act(key, sq)
                        for half in range(2):
                            ps = ps_s1.tile([128, 512], F32, tag="ps")
                            for c in range(NDM):
                                nc.tensor.matmul(
                                    ps[:],
                                    wts[key][:, c, 0:128],
                                    act[:, c, half * 512 : half * 512 + 512],
                                    start=(c == 0),
                                    stop=(c == NDM - 1),
                                )
                            nc.vector.tensor_scalar_add(
                                dsts[key][:, 0, sq * 1024 + half * 512 : sq * 1024 + half * 512 + 512],
                                ps[:],
                                biases[key][:, 0:1],
                            )
                # V projection (bias via ones-row matmul)
                for sq in range(NQB):
                    act = load_act("v", sq)
                    for sti in range(8):
                        st = sq * 8 + sti
                        ps = ps_s1.tile([128, 512], F32, tag="ps")
                        for c in range(NDM):
                            nc.tensor.matmul(
                                ps[:],
                                act[:, c, sti * 128 : (sti + 1) * 128],
                                wts["v"][:, c, :],
                                start=(c == 0),
                                stop=False,
                            )
                        nc.tensor.matmul(
                            ps[:], ones_sb[0:1, :], bv_sb[0:1, :], start=False, stop=True
                        )
                        nc.vector.tensor_copy(
                            Vp[:, st, :, 0:DV],
                            ps[:].rearrange("p (h d) -> p h d", h=HL),
                        )

            if DEBUG_DUMP:
                nc.sync.dma_start(vp_dbg[:], Vp[:])

            # ---- Stage 2: attention with interleaved t=1..3 projections ----
            with tc.tile_pool(name="att", bufs=1) as att_pool:
                AT = att_pool.tile([128, NT, S], BF16)
                with (
                    tc.tile_pool(name="expS", bufs=5) as exp_pool,
                    tc.tile_pool(name="rec", bufs=2) as rec_pool,
                    tc.tile_pool(name="ps_sc", bufs=3, space="PSUM") as ps_sc,
                    tc.tile_pool(name="ps_pv", bufs=2, space="PSUM") as ps_pv,
                ):
                    for hp in range(HL // 2):  # head pairs: even head on
                        t = hp                 # partitions 0-63, odd on 64-127

                        # per-hp projection work for row-tile t=hp+1:
                        # 8 chains of 8 matmuls, one matmul per tick (64 ticks)
                        chains = []
                        if hp < NT - 1:
                            tn = hp + 1
                            for key in ("q", "k"):
                                for sq in range(NQB):
                                    for half in range(2):
                                        chains.append((key, tn, sq, half))
                        chain_ps = [None]
                        chain_act = {}

                        def proj_tick(tick):
                            ci, step = tick // 8, tick % 8
                            if ci >= len(chains):
                                return
                            key, tn, sq, half = chains[ci]
                            if step == 0 and (key, sq) not in chain_act:
                                chain_act.clear()
                                chain_act[(key, sq)] = load_act(key, sq)
                            act = chain_act[(key, sq)]
                            if step == 0:
                                chain_ps[0] = ps_sc.tile(
                                    [128, 2, 512], F32, tag="sc", name="projps"
                                )
                            cps = chain_ps[0]
                            nc.tensor.matmul(
                                cps[:, 0, :],
                                wts[key][:, step, tn * 128 : (tn + 1) * 128],
                                act[:, step, half * 512 : half * 512 + 512],
                                start=(step == 0),
                                stop=(step == NDM - 1),
                            )
                            if step == NDM - 1:
                                nc.vector.tensor_scalar_add(
                                    dsts[key][
                                        :,
                                        tn,
                                        sq * 1024 + half * 512 : sq * 1024 + half * 512 + 512,
                                    ],
                                    cps[:, 0, :],
                                    biases[key][:, tn : tn + 1],
                                )

                        tick = 0
                        for qb in range(4):  # q-blocks of 512
                            qsl = slice(qb * 512, (qb + 1) * 512)
                            pv_list = [
                                ps_pv.tile([128, 512], F32, tag="pv", name=f"pv{i}")
                                for i in range(2)
                            ]

                            def emit_pv(kt, ex):
                                for sub in range(2):
                                    nc.tensor.matmul(
                                        pv_list[sub][0 : DV + 1, :],
                                        Vp[:, kt, hp * 2 + sub, :],
                                        ex[:, sub, :],
                                        start=(kt == 0),
                                        stop=(kt == NKT - 1),
                                    )

                            ex_tiles = []
                            for kt in range(NKT):
                                scp = ps_sc.tile([128, 2, 512], F32, tag="sc")
                                for sub in range(2):
                                    psl = slice(sub * 64, sub * 64 + 64)
                                    nc.tensor.matmul(
                                        scp[:, sub, :],
                                        KT[psl, t, kt * 128 : (kt + 1) * 128],
                                        QT[psl, t, qsl],
                                        start=True,
                                        stop=True,
                                    )
                                ex = exp_pool.tile([128, 2, 512], BF16, tag="e")
                                nc.scalar.activation(ex[:], scp[:], AF.Exp, scale=SCALE)
                                if DEBUG_DUMP and hp == 0 and qb == 0 and kt == 0:
                                    nc.sync.dma_start(ex_dbg[:], ex[:, 0, :])
                                ex_tiles.append(ex)
                                if kt > 0:
                                    emit_pv(kt - 1, ex_tiles[kt - 1])
                                proj_tick(tick)
                                tick += 1
                            emit_pv(NKT - 1, ex_tiles[NKT - 1])
                            if DEBUG_DUMP and hp == 0 and qb == 0:
                                pv_stg = rec_pool.tile([128, 512], F32, tag="pvstg")
                                nc.vector.tensor_copy(pv_stg[:], pv_list[0][:])
                                nc.sync.dma_start(pv_dbg[:], pv_stg[:])
                            for sub in range(2):
                                psl = slice(sub * 64, sub * 64 + 64)
                                pvp = pv_list[sub]
                                rec = rec_pool.tile([1, 512], F32, tag="r")
                                recb = rec_pool.tile([64, 512], F32, tag="rb")
                                dcp = rec_pool.tile([1, 512], F32, tag="d")
                                # custom-DVE ucode mishandles base_partition=64
                                # PSUM reads; stage through partition 0
                                nc.vector.tensor_copy(dcp[:], pvp[DV : DV + 1, :])
                                nc.vector.reciprocal_approx_fast(rec[:], dcp[:])
                                nc.gpsimd.partition_broadcast(recb[:], rec[:])
                                nc.vector.tensor_mul(
                                    AT[psl, t, qsl], pvp[0:DV, :], recb[:]
                                )

                if DEBUG_DUMP:
                    nc.sync.dma_start(at_dbg[:], AT[:])
                    nc.sync.dma_start(qt_dbg[:], QT[:])
                    nc.sync.dma_start(kt_dbg[:], KT[:])

                # ---- Stage 3: output projection ----
                with (
                    tc.tile_pool(name="ostage", bufs=3) as ostage,
                    tc.tile_pool(name="ps_o", bufs=2, space="PSUM") as ps_o,
                ):
                    for m in range(NDM):
                        for sbk in range(NQB):
                            ps = ps_o.tile([128, 1024], F32, tag="po")
                            for half in range(2):
                                hs = slice(
                                    sbk * 1024 + half * 512,
                                    sbk * 1024 + half * 512 + 512,
                                )
                                for cc in range(NT):
                                    nc.tensor.matmul(
                                        ps[:, half * 512 : half * 512 + 512],
                                        wo_sb[:, cc, m * 128 : (m + 1) * 128],
                                        AT[:, cc, hs],
                                        start=(cc == 0),
                                        stop=(cc == NT - 1),
                                    )
                            ot = ostage.tile([128, 1024], F32, tag="o")
                            nc.vector.tensor_copy(ot[:], ps[:])
                            nc.sync.dma_start(
                                outT[
                                    m * 128 : (m + 1) * 128,
                                    sbk * 1024 : (sbk + 1) * 1024,
                                ],
                                ot[:],
                            )
            wgt_pool.release()
            acts_pool.release()

    nc.compile()
    return nc


def get_nc():
    global _CACHED_NC
    if _CACHED_NC is None:
        _CACHED_NC = _build()
    return _CACHED_NC


def _bf(x):
    return np.ascontiguousarray(np.asarray(x, np.float32)).astype(NP_BF16)


def make_in_maps(queries, keys, values, Wq, bq, Wk, bk, Wv, bv, Wo, bo):
    queries = np.asarray(queries, np.float32)
    keys = np.asarray(keys, np.float32)
    values = np.asarray(values, np.float32)
    Wq = np.asarray(Wq, np.float32)
    Wk = np.asarray(Wk, np.float32)
    Wv = np.asarray(Wv, np.float32)
    Wo = np.asarray(Wo, np.float32)
    bq = np.asarray(bq, np.float32)
    bk = np.asarray(bk, np.float32)
    bv = np.asarray(bv, np.float32)
    ones = np.ones((1, 128), NP_BF16)
    in_maps = []
    for core in range(8):
        b, hg = divmod(core, 2)
        sl = slice(hg * NB, (hg + 1) * NB)
        in_maps.append(
            {
                "qT": _bf(queries[b].T),
                "kT": _bf(keys[b].T),
                "vT": _bf(values[b].T),
                "wq": _bf(Wq[:, sl]),
                "wk": _bf(Wk[:, sl]),
                "wv": _bf(Wv[:, sl]),
                "wo": _bf(Wo[sl, :]),
                "bq": np.ascontiguousarray(bq[sl]),
                "bk": np.ascontiguousarray(bk[sl]),
                "bv": _bf(bv[sl]),
                "ones": ones,
            }
        )
    return in_maps


def assemble(results, bo):
    bo = np.asarray(bo, np.float32)
    out = np.empty((B, S, DM), np.float32)
    for b in range(B):
        acc = results[2 * b]["outT"] + results[2 * b + 1]["outT"]
        out[b] = acc.T + bo
    return out


def run(trace=False, **inputs):
    if trace:
        # NTFF profiling shim: this image's antenv lacks axon_hooks.
        import types

        try:
            from antenv import axon_hooks  # noqa: F401
        except ImportError:
            from trn_agent_boot.trn_boot import _ntff_profile_via_ctypes

            mod = types.ModuleType("antenv.axon_hooks")
            _hook = _ntff_profile_via_ctypes("/opt/axon/libaxon_pjrt.so")
            mod.get_axon_ntff_profile_hook = lambda: _hook
            sys.modules["antenv.axon_hooks"] = mod
    nc = get_nc()
    bo = inputs["bo"]
    in_maps = make_in_maps(**inputs)
    res = run_bass_kernel_spmd(nc, in_maps, list(range(8)), trace=trace)
    return assemble(res.results, bo), res


def kernel(**inputs):
    out, _ = run(trace=False, **inputs)
    return out


# revision 23
# speedup vs baseline: 1.2994x; 1.0430x over previous
"""Multi-head attention on 8 Trainium2 NeuronCores (Bass/Tile).

Sharding: batch B=4 x head-groups 2 -> 8 cores. Each core computes full
attention for 1 batch element and 8 of 16 heads, producing a partial
output projection (Wo row-sharded); host sums the two partials per batch.

Device dataflow (per core), everything in "transposed" orientation so the
contraction dim always sits on SBUF partitions. All matmul operands are
bf16 (fp32 PSUM accumulation); fp32r measured 3x slower per column on HW.
  qT/kT/vT [DM=1024, S=2048] bf16 from host (host pre-transposes+casts).
  QT = (Wq^T qT) [512, S] bf16  (DVE adds bq while copying PSUM->SBUF)
  KT likewise; V natural [S, 512] via lhsT=vT s-tiles (bv added with a
  K=1 ones-row matmul), stored per head with an appended ones column:
  Vp [k-tile, head, 65] bf16.
  scores^T[k,q] = (K_h^T tile).T @ Q_h^T, N=1024 (K=64 contraction; even/
  odd heads on partitions 0-63/64-127 -> concurrent PE row-tiles).
  expS = Exp(scores * 0.125) ACT PSUM->SBUF bf16, [128,1024] chunks.
  PV: out'[65, q] += Vp_tile.T @ expS_tile accumulated over 16 k-tiles;
  row 64 is the softmax denominator (ones column trick).
  A^T = out'[0:64] * reciprocal(out'[64]) broadcast -> bf16 [512, S].
  O^T[m, s] = Wo_chunk.T @ A^T chunk accumulated over 4 chunks -> fp32.
Host: out[b] = (O^T_hg0 + O^T_hg1).T + bo.
"""

import sys

sys.path.insert(0, "/opt/trn_rl_repo")

import ml_dtypes
import numpy as np

import concourse.bacc as bacc
import concourse.mybir as mybir
from concourse import tile
from concourse.bass_utils import run_bass_kernel_spmd

F32 = mybir.dt.float32
BF16 = mybir.dt.bfloat16
AF = mybir.ActivationFunctionType
NP_BF16 = ml_dtypes.bfloat16

H, DK, DV, DM = 16, 64, 64, 1024
B, S = 4, 2048
HL = H // 2          # heads per core
NB = HL * DK         # 512: per-core projection width
NDM = DM // 128      # 8 contraction chunks
NT = NB // 128       # 4 row-tiles of QT/KT/AT
NKT = S // 128       # 16 k-tiles
NQB = S // 1024      # 2 q-blocks of 1024
SCALE = 1.0 / 8.0    # 1/sqrt(DK)

_CACHED_NC = None

import os

DEBUG_DUMP = bool(os.environ.get("KERNEL_DEBUG_DUMP"))


def _build():
    nc = bacc.Bacc("TRN2", debug=False)

    qT = nc.dram_tensor("qT", [DM, S], BF16, kind="ExternalInput")
    kT = nc.dram_tensor("kT", [DM, S], BF16, kind="ExternalInput")
    vT = nc.dram_tensor("vT", [DM, S], BF16, kind="ExternalInput")
    wq = nc.dram_tensor("wq", [DM, NB], BF16, kind="ExternalInput")
    wk = nc.dram_tensor("wk", [DM, NB], BF16, kind="ExternalInput")
    wv = nc.dram_tensor("wv", [DM, NB], BF16, kind="ExternalInput")
    wo = nc.dram_tensor("wo", [NB, DM], BF16, kind="ExternalInput")
    bq = nc.dram_tensor("bq", [NB], F32, kind="ExternalInput")
    bk = nc.dram_tensor("bk", [NB], F32, kind="ExternalInput")
    bv = nc.dram_tensor("bv", [NB], BF16, kind="ExternalInput")
    ones = nc.dram_tensor("ones", [1, 128], BF16, kind="ExternalInput")
    outT = nc.dram_tensor("outT", [DM, S], BF16, kind="ExternalOutput")
    if DEBUG_DUMP:
        qt_dbg = nc.dram_tensor("qt_dbg", [128, NT, S], BF16, kind="ExternalOutput")
        kt_dbg = nc.dram_tensor("kt_dbg", [128, NT, S], BF16, kind="ExternalOutput")
        vp_dbg = nc.dram_tensor(
            "vp_dbg", [128, NKT, HL, DV + 1], BF16, kind="ExternalOutput"
        )
        at_dbg = nc.dram_tensor("at_dbg", [128, NT, S], BF16, kind="ExternalOutput")
        ex_dbg = nc.dram_tensor("ex_dbg", [128, 512], BF16, kind="ExternalOutput")
        pv_dbg = nc.dram_tensor("pv_dbg", [128, 512], F32, kind="ExternalOutput")

    with tile.TileContext(nc) as tc:
        with tc.tile_pool(name="persist", bufs=1) as persist:
            QT = persist.tile([128, NT, S], BF16)
            KT = persist.tile([128, NT, S], BF16)
            Vp = persist.tile([128, NKT, HL, DV + 1], BF16)
            wo_sb = persist.tile([128, NT, DM], BF16)
            bq_sb = persist.tile([128, NT], F32)
            bk_sb = persist.tile([128, NT], F32)
            bv_sb = persist.tile([1, NB], BF16)
            ones_sb = persist.tile([1, 128], BF16)

            nc.vector.memset(Vp[:, :, :, DV : DV + 1], 1.0)

            # ---- Stage 1: V projection + Q/K projections for t=0 ----
            # Q/K projections for t=1..3 are interleaved into the attention
            # loop below (one matmul per k-tile tick) so they fill the PE
            # while ACT paces the softmax and the HAM clock stays warm.
            acts_pool = tc.alloc_tile_pool(name="acts", bufs=2)
            wgt_pool = tc.alloc_tile_pool(name="wgt", bufs=1)

            wts = {}

            def load_w(key, src_w):
                wt = wgt_pool.tile([128, NDM, NB], BF16, tag=f"w{key}", name=f"w{key}")
                for c in range(NDM):
                    nc.sync.dma_start(wt[:, c, :], src_w[c * 128 : (c + 1) * 128, :])
                wts[key] = wt

            w_srcs = {"q": wq, "k": wk, "v": wv}
            srcs = {"q": qT, "k": kT, "v": vT}
            biases = {"q": bq_sb, "k": bk_sb}
            dsts = {"q": QT, "k": KT}

            def load_act(key, sq):
                act = acts_pool.tile([128, NDM, 1024], BF16, tag="a", name=f"a{key}{sq}")
                for c in range(NDM):
                    nc.sync.dma_start(
                        act[:, c, :],
                        srcs[key][
                            c * 128 : (c + 1) * 128, sq * 1024 : (sq + 1) * 1024
                        ],
                    )
                return act

            with tc.tile_pool(name="ps_s1", bufs=4, space="PSUM") as ps_s1:
                # Q/K projections for t=0
                for key in ("q", "k"):
                    load_w(key, w_srcs[key])
                    nc.sync.dma_start(
                        biases[key][:],
                        (bq if key == "q" else bk).rearrange("(t p) -> p t", p=128),
                    )
                    for sq in range(NQB):
                        act = load_act(key, sq)
                        for half in range(2):
                            ps = ps_s1.tile([128, 512], F32, tag="ps")
                            for c in range(NDM):
                                nc.tensor.matmul(
                                    ps[:],
                                    wts[key][:, c, 0:128],
                                    act[:, c, half * 512 : half * 512 + 512],
                                    start=(c == 0),
                                    stop=(c == NDM - 1),
                                )
                            nc.vector.tensor_scalar_add(
                                dsts[key][:, 0, sq * 1024 + half * 512 : sq * 1024 + half * 512 + 512],
                                ps[:],
                                biases[key][:, 0:1],
                            )
                # V projection (bias via ones-row matmul)
                load_w("v", w_srcs["v"])
                nc.sync.dma_start(bv_sb[:], bv.rearrange("(o n) -> o n", o=1))
                nc.sync.dma_start(ones_sb[:], ones[:])
                for sq in range(NQB):
                    act = load_act("v", sq)
                    for sti in range(8):
                        st = sq * 8 + sti
                        ps = ps_s1.tile([128, 512], F32, tag="ps")
                        for c in range(NDM):
                            nc.tensor.matmul(
                                ps[:],
                                act[:, c, sti * 128 : (sti + 1) * 128],
                                wts["v"][:, c, :],
                                start=(c == 0),
                                stop=False,
                            )
                        nc.tensor.matmul(
                            ps[:], ones_sb[0:1, :], bv_sb[0:1, :], start=False, stop=True
                        )
                        nc.vector.tensor_copy(
                            Vp[:, st, :, 0:DV],
                            ps[:].rearrange("p (h d) -> p h d", h=HL),
                        )

            if DEBUG_DUMP:
                nc.sync.dma_start(vp_dbg[:], Vp[:])

            # ---- Stage 2: attention with interleaved t=1..3 projections ----
            with tc.tile_pool(name="att", bufs=1) as att_pool:
                AT = att_pool.tile([128, NT, S], BF16)
                with (
                    tc.tile_pool(name="expS", bufs=5) as exp_pool,
                    tc.tile_pool(name="rec", bufs=2) as rec_pool,
                    tc.tile_pool(name="ps_sc", bufs=2, space="PSUM") as ps_sc,
                    tc.tile_pool(name="ps_pv", bufs=4, space="PSUM") as ps_pv,
                ):
                    for hp in range(HL // 2):  # head pairs: even head on
                        t = hp                 # partitions 0-63, odd on 64-127

                        # per-hp projection work for row-tile t=hp+1:
                        # 8 chains of 8 matmuls, one matmul per tick (64 ticks)
                        chains = []
                        if hp < NT - 1:
                            tn = hp + 1
                            for key in ("q", "k"):
                                for sq in range(NQB):
                                    for half in range(2):
                                        chains.append((key, tn, sq, half))
                        chain_ps = [None]
                        chain_act = {}

                        def proj_tick(tick):
                            ci, step = tick // 8, tick % 8
                            if ci >= len(chains):
                                return
                            key, tn, sq, half = chains[ci]
                            if step == 0 and (key, sq) not in chain_act:
                                chain_act.clear()
                                chain_act[(key, sq)] = load_act(key, sq)
                            act = chain_act[(key, sq)]
                            if step == 0:
                                chain_ps[0] = ps_pv.tile(
                                    [128, 512], F32, tag="pv", name="projps"
                                )
                            cps = chain_ps[0]
                            nc.tensor.matmul(
                                cps[:],
                                wts[key][:, step, tn * 128 : (tn + 1) * 128],
                                act[:, step, half * 512 : half * 512 + 512],
                                start=(step == 0),
                                stop=(step == NDM - 1),
                            )
                            if step == NDM - 1:
                                nc.vector.tensor_scalar_add(
                                    dsts[key][
                                        :,
                                        tn,
                                        sq * 1024 + half * 512 : sq * 1024 + half * 512 + 512,
                                    ],
                                    cps[:],
                                    biases[key][:, tn : tn + 1],
                                )

                        tick = 0
                        for qb in range(4):  # q-blocks of 512
                            qsl = slice(qb * 512, (qb + 1) * 512)
                            pv_list = [
                                ps_pv.tile([128, 512], F32, tag="pv", name=f"pv{i}")
                                for i in range(2)
                            ]

                            def emit_pv(kt, ex):
                                for sub in range(2):
                                    nc.tensor.matmul(
                                        pv_list[sub][0 : DV + 1, :],
                                        Vp[:, kt, hp * 2 + sub, :],
                                        ex[:, sub, :],
                                        start=(kt == 0),
                                        stop=(kt == NKT - 1),
                                    )

                            ex_tiles = []
                            for kt in range(NKT):
                                scp = ps_sc.tile([128, 2, 512], F32, tag="sc")
                                for sub in range(2):
                                    psl = slice(sub * 64, sub * 64 + 64)
                                    nc.tensor.matmul(
                                        scp[:, sub, :],
                                        KT[psl, t, kt * 128 : (kt + 1) * 128],
                                        QT[psl, t, qsl],
                                        start=True,
                                        stop=True,
                                    )
                                ex = exp_pool.tile([128, 2, 512], BF16, tag="e")
                                nc.scalar.activation(ex[:], scp[:], AF.Exp, scale=SCALE)
                                if DEBUG_DUMP and hp == 0 and qb == 0 and kt == 0:
                                    nc.sync.dma_start(ex_dbg[:], ex[:, 0, :])
                                ex_tiles.append(ex)
                                if kt > 0:
                                    emit_pv(kt - 1, ex_tiles[kt - 1])
                                proj_tick(tick)
                                tick += 1
                            emit_pv(NKT - 1, ex_tiles[NKT - 1])
                            if DEBUG_DUMP and hp == 0 and qb == 0:
                                pv_stg = rec_pool.tile([128, 512], F32, tag="pvstg")
                                nc.vector.tensor_copy(pv_stg[:], pv_list[0][:])
                                nc.sync.dma_start(pv_dbg[:], pv_stg[:])
                            for sub in range(2):
                                psl = slice(sub * 64, sub * 64 + 64)
                                pvp = pv_list[sub]
                                rec = rec_pool.tile([1, 512], F32, tag="r")
                                recb = rec_pool.tile([64, 512], F32, tag="rb")
                                dcp = rec_pool.tile([1, 512], F32, tag="d")
                                # custom-DVE ucode mishandles base_partition=64
                                # PSUM reads; stage through partition 0
                                nc.vector.tensor_copy(dcp[:], pvp[DV : DV + 1, :])
                                nc.vector.reciprocal_approx_fast(rec[:], dcp[:])
                                nc.gpsimd.partition_broadcast(recb[:], rec[:])
                                nc.vector.tensor_mul(
                                    AT[psl, t, qsl], pvp[0:DV, :], recb[:]
                                )

                if DEBUG_DUMP:
                    nc.sync.dma_start(at_dbg[:], AT[:])
                    nc.sync.dma_start(qt_dbg[:], QT[:])
                    nc.sync.dma_start(kt_dbg[:], KT[:])

                # ---- Stage 3: output projection ----
                with (
                    tc.tile_pool(name="ostage", bufs=3) as ostage,
                    tc.tile_pool(name="ps_o", bufs=2, space="PSUM") as ps_o,
                ):
                    for c in range(NT):
                        nc.sync.dma_start(
                            wo_sb[:, c, :], wo[c * 128 : (c + 1) * 128, :]
                        )
                    for m in range(NDM):
                        for sbk in range(NQB):
                            ps = ps_o.tile([128, 1024], F32, tag="po")
                            for half in range(2):
                                hs = slice(
                                    sbk * 1024 + half * 512,
                                    sbk * 1024 + half * 512 + 512,
                                )
                                for cc in range(NT):
                                    nc.tensor.matmul(
                                        ps[:, half * 512 : half * 512 + 512],
                                        wo_sb[:, cc, m * 128 : (m + 1) * 128],
                                        AT[:, cc, hs],
                                        start=(cc == 0),
                                        stop=(cc == NT - 1),
                                    )
                            ot = ostage.tile([128, 1024], BF16, tag="o")
                            nc.scalar.copy(ot[:], ps[:])
                            nc.sync.dma_start(
                                outT[
                                    m * 128 : (m + 1) * 128,
                                    sbk * 1024 : (sbk + 1) * 1024,
                                ],
                                ot[:],
                            )
            wgt_pool.release()
            acts_pool.release()

    nc.compile()
    return nc


def get_nc():
    global _CACHED_NC
    if _CACHED_NC is None:
        _CACHED_NC = _build()
    return _CACHED_NC


def _bf(x):
    return np.ascontiguousarray(np.asarray(x, np.float32)).astype(NP_BF16)


def make_in_maps(queries, keys, values, Wq, bq, Wk, bk, Wv, bv, Wo, bo):
    queries = np.asarray(queries, np.float32)
    keys = np.asarray(keys, np.float32)
    values = np.asarray(values, np.float32)
    Wq = np.asarray(Wq, np.float32)
    Wk = np.asarray(Wk, np.float32)
    Wv = np.asarray(Wv, np.float32)
    Wo = np.asarray(Wo, np.float32)
    bq = np.asarray(bq, np.float32)
    bk = np.asarray(bk, np.float32)
    bv = np.asarray(bv, np.float32)
    ones = np.ones((1, 128), NP_BF16)
    in_maps = []
    for core in range(8):
        b, hg = divmod(core, 2)
        sl = slice(hg * NB, (hg + 1) * NB)
        in_maps.append(
            {
                "qT": _bf(queries[b].T),
                "kT": _bf(keys[b].T),
                "vT": _bf(values[b].T),
                "wq": _bf(Wq[:, sl]),
                "wk": _bf(Wk[:, sl]),
                "wv": _bf(Wv[:, sl]),
                "wo": _bf(Wo[sl, :]),
                "bq": np.ascontiguousarray(bq[sl]),
                "bk": np.ascontiguousarray(bk[sl]),
                "bv": _bf(bv[sl]),
                "ones": ones,
            }
        )
    return in_maps


def assemble(results, bo):
    bo = np.asarray(bo, np.float32)
    out = np.empty((B, S, DM), np.float32)
    for b in range(B):
        acc = np.asarray(results[2 * b]["outT"], np.float32) + np.asarray(
            results[2 * b + 1]["outT"], np.float32
        )
        out[b] = acc.T + bo
    return out


def run(trace=False, **inputs):
    if trace:
        # NTFF profiling shim: this image's antenv lacks axon_hooks.
        import types

        try:
            from antenv import axon_hooks  # noqa: F401
        except ImportError:
            from trn_agent_boot.trn_boot import _ntff_profile_via_ctypes

            mod = types.ModuleType("antenv.axon_hooks")
            _hook = _ntff_profile_via_ctypes("/opt/axon/libaxon_pjrt.so")
            mod.get_axon_ntff_profile_hook = lambda: _hook
            sys.modules["antenv.axon_hooks"] = mod
    nc = get_nc()
    bo = inputs["bo"]
    in_maps = make_in_maps(**inputs)
    res = run_bass_kernel_spmd(nc, in_maps, list(range(8)), trace=trace)
    return assemble(res.results, bo), res


def kernel(**inputs):
    out, _ = run(trace=False, **inputs)
    return out


# revision 24
# speedup vs baseline: 1.3502x; 1.0391x over previous
"""Multi-head attention on 8 Trainium2 NeuronCores (Bass/Tile).

Sharding: batch B=4 x head-groups 2 -> 8 cores. Each core computes full
attention for 1 batch element and 8 of 16 heads, producing a partial
output projection (Wo row-sharded); host sums the two partials per batch.

Device dataflow (per core), everything in "transposed" orientation so the
contraction dim always sits on SBUF partitions. All matmul operands are
bf16 (fp32 PSUM accumulation); fp32r measured 3x slower per column on HW.
  qT/kT/vT [DM=1024, S=2048] bf16 from host (host pre-transposes+casts).
  QT = (Wq^T qT) [512, S] bf16  (DVE adds bq while copying PSUM->SBUF)
  KT likewise; V natural [S, 512] via lhsT=vT s-tiles (bv added with a
  K=1 ones-row matmul), stored per head with an appended ones column:
  Vp [k-tile, head, 65] bf16.
  scores^T[k,q] = (K_h^T tile).T @ Q_h^T, N=1024 (K=64 contraction; even/
  odd heads on partitions 0-63/64-127 -> concurrent PE row-tiles).
  expS = Exp(scores * 0.125) ACT PSUM->SBUF bf16, [128,1024] chunks.
  PV: out'[65, q] += Vp_tile.T @ expS_tile accumulated over 16 k-tiles;
  row 64 is the softmax denominator (ones column trick).
  A^T = out'[0:64] * reciprocal(out'[64]) broadcast -> bf16 [512, S].
  O^T[m, s] = Wo_chunk.T @ A^T chunk accumulated over 4 chunks -> fp32.
Host: out[b] = (O^T_hg0 + O^T_hg1).T + bo.
"""

import sys

sys.path.insert(0, "/opt/trn_rl_repo")

import ml_dtypes
import numpy as np

import concourse.bacc as bacc
import concourse.mybir as mybir
from concourse import tile
from concourse.bass_utils import run_bass_kernel_spmd

F32 = mybir.dt.float32
BF16 = mybir.dt.bfloat16
AF = mybir.ActivationFunctionType
NP_BF16 = ml_dtypes.bfloat16

H, DK, DV, DM = 16, 64, 64, 1024
B, S = 4, 2048
HL = H // 2          # heads per core
NB = HL * DK         # 512: per-core projection width
NDM = DM // 128      # 8 contraction chunks
NT = NB // 128       # 4 row-tiles of QT/KT/AT
NKT = S // 128       # 16 k-tiles
NQB = S // 1024      # 2 q-blocks of 1024
SCALE = 1.0 / 8.0    # 1/sqrt(DK)

_CACHED_NC = None

import os

DEBUG_DUMP = bool(os.environ.get("KERNEL_DEBUG_DUMP"))


def _build():
    nc = bacc.Bacc("TRN2", debug=False)

    qT = nc.dram_tensor("qT", [DM, S], BF16, kind="ExternalInput")
    kT = nc.dram_tensor("kT", [DM, S], BF16, kind="ExternalInput")
    vT = nc.dram_tensor("vT", [DM, S], BF16, kind="ExternalInput")
    wq = nc.dram_tensor("wq", [DM, NB], BF16, kind="ExternalInput")
    wk = nc.dram_tensor("wk", [DM, NB], BF16, kind="ExternalInput")
    wv = nc.dram_tensor("wv", [DM, NB], BF16, kind="ExternalInput")
    wo = nc.dram_tensor("wo", [NB, DM], BF16, kind="ExternalInput")
    bq = nc.dram_tensor("bq", [NB], F32, kind="ExternalInput")
    bk = nc.dram_tensor("bk", [NB], F32, kind="ExternalInput")
    bv = nc.dram_tensor("bv", [NB], BF16, kind="ExternalInput")
    ones = nc.dram_tensor("ones", [1, 128], BF16, kind="ExternalInput")
    outT = nc.dram_tensor("outT", [DM, S], BF16, kind="ExternalOutput")
    if DEBUG_DUMP:
        qt_dbg = nc.dram_tensor("qt_dbg", [128, NT, S], BF16, kind="ExternalOutput")
        kt_dbg = nc.dram_tensor("kt_dbg", [128, NT, S], BF16, kind="ExternalOutput")
        vp_dbg = nc.dram_tensor(
            "vp_dbg", [128, NKT, HL, DV + 1], BF16, kind="ExternalOutput"
        )
        at_dbg = nc.dram_tensor("at_dbg", [128, NT, S], BF16, kind="ExternalOutput")
        ex_dbg = nc.dram_tensor("ex_dbg", [128, 512], BF16, kind="ExternalOutput")
        pv_dbg = nc.dram_tensor("pv_dbg", [128, 512], F32, kind="ExternalOutput")

    with tile.TileContext(nc) as tc:
        with tc.tile_pool(name="persist", bufs=1) as persist:
            QT = persist.tile([128, NT, S], BF16)
            KT = persist.tile([128, NT, S], BF16)
            Vp = persist.tile([128, NKT, HL, DV + 1], BF16)
            wo_sb = persist.tile([128, NT, DM], BF16)
            bq_sb = persist.tile([128, NT], F32)
            bk_sb = persist.tile([128, NT], F32)
            bv_sb = persist.tile([1, NB], BF16)
            ones_sb = persist.tile([1, 128], BF16)

            nc.vector.memset(Vp[:, :, :, DV : DV + 1], 1.0)

            # ---- Stage 1: V projection + Q/K projections for t=0 ----
            # Q/K projections for t=1..3 are interleaved into the attention
            # loop below (one matmul per k-tile tick) so they fill the PE
            # while ACT paces the softmax and the HAM clock stays warm.
            acts_pool = tc.alloc_tile_pool(name="acts", bufs=2)
            wgt_pool = tc.alloc_tile_pool(name="wgt", bufs=1)

            wts = {}

            def load_w(key, src_w):
                wt = wgt_pool.tile([128, NDM, NB], BF16, tag=f"w{key}", name=f"w{key}")
                for c in range(NDM):
                    nc.sync.dma_start(wt[:, c, :], src_w[c * 128 : (c + 1) * 128, :])
                wts[key] = wt

            w_srcs = {"q": wq, "k": wk, "v": wv}
            srcs = {"q": qT, "k": kT, "v": vT}
            biases = {"q": bq_sb, "k": bk_sb}
            dsts = {"q": QT, "k": KT}

            def load_act(key, sq):
                act = acts_pool.tile([128, NDM, 1024], BF16, tag="a", name=f"a{key}{sq}")
                for c in range(NDM):
                    nc.sync.dma_start(
                        act[:, c, :],
                        srcs[key][
                            c * 128 : (c + 1) * 128, sq * 1024 : (sq + 1) * 1024
                        ],
                    )
                return act

            with tc.tile_pool(name="ps_s1", bufs=4, space="PSUM") as ps_s1:
                # V projection first (vT is the only DMA the head waits on;
                # qT/kT stream in underneath the V matmuls)
                load_w("v", w_srcs["v"])
                nc.sync.dma_start(bv_sb[:], bv.rearrange("(o n) -> o n", o=1))
                nc.sync.dma_start(ones_sb[:], ones[:])
                for sq in range(NQB):
                    act = load_act("v", sq)
                    for sti in range(8):
                        st = sq * 8 + sti
                        ps = ps_s1.tile([128, 512], F32, tag="ps")
                        for c in range(NDM):
                            nc.tensor.matmul(
                                ps[:],
                                act[:, c, sti * 128 : (sti + 1) * 128],
                                wts["v"][:, c, :],
                                start=(c == 0),
                                stop=False,
                            )
                        nc.tensor.matmul(
                            ps[:], ones_sb[0:1, :], bv_sb[0:1, :], start=False, stop=True
                        )
                        nc.vector.tensor_copy(
                            Vp[:, st, :, 0:DV],
                            ps[:].rearrange("p (h d) -> p h d", h=HL),
                        )
                # Q/K projections for t=0
                for key in ("q", "k"):
                    load_w(key, w_srcs[key])
                    nc.sync.dma_start(
                        biases[key][:],
                        (bq if key == "q" else bk).rearrange("(t p) -> p t", p=128),
                    )
                    for sq in range(NQB):
                        act = load_act(key, sq)
                        for half in range(2):
                            ps = ps_s1.tile([128, 512], F32, tag="ps")
                            for c in range(NDM):
                                nc.tensor.matmul(
                                    ps[:],
                                    wts[key][:, c, 0:128],
                                    act[:, c, half * 512 : half * 512 + 512],
                                    start=(c == 0),
                                    stop=(c == NDM - 1),
                                )
                            nc.vector.tensor_scalar_add(
                                dsts[key][:, 0, sq * 1024 + half * 512 : sq * 1024 + half * 512 + 512],
                                ps[:],
                                biases[key][:, 0:1],
                            )

            if DEBUG_DUMP:
                nc.sync.dma_start(vp_dbg[:], Vp[:])

            # ---- Stage 2: attention with interleaved t=1..3 projections ----
            with tc.tile_pool(name="att", bufs=1) as att_pool:
                AT = att_pool.tile([128, NT, S], BF16)
                with (
                    tc.tile_pool(name="expS", bufs=5) as exp_pool,
                    tc.tile_pool(name="rec", bufs=2) as rec_pool,
                    tc.tile_pool(name="ps_sc", bufs=2, space="PSUM") as ps_sc,
                    tc.tile_pool(name="ps_pv", bufs=4, space="PSUM") as ps_pv,
                ):
                    # Flat column stream over (hp, qb, kt) with the PV
                    # matmuls skewed one column behind scores/exp — including
                    # across unit boundaries — so the PE never stalls on the
                    # exp of the column it just produced and the ACT stream
                    # stays gapless.
                    pv_store = {}
                    proj_fns = {}
                    for hp in range(HL // 2):
                        chains = []
                        if hp < NT - 1:
                            tn = hp + 1
                            for key in ("q", "k"):
                                for sq in range(NQB):
                                    for half in range(2):
                                        chains.append((key, tn, sq, half))
                        chain_ps = [None]
                        chain_act = {}

                        def make_proj_tick(chains, chain_ps, chain_act):
                            def proj_tick(tick):
                                ci, step = tick // 8, tick % 8
                                if ci >= len(chains):
                                    return
                                key, tn, sq, half = chains[ci]
                                if step == 0 and (key, sq) not in chain_act:
                                    chain_act.clear()
                                    chain_act[(key, sq)] = load_act(key, sq)
                                act = chain_act[(key, sq)]
                                if step == 0:
                                    chain_ps[0] = ps_pv.tile(
                                        [128, 512], F32, tag="pv", name="projps"
                                    )
                                cps = chain_ps[0]
                                nc.tensor.matmul(
                                    cps[:],
                                    wts[key][:, step, tn * 128 : (tn + 1) * 128],
                                    act[:, step, half * 512 : half * 512 + 512],
                                    start=(step == 0),
                                    stop=(step == NDM - 1),
                                )
                                if step == NDM - 1:
                                    nc.vector.tensor_scalar_add(
                                        dsts[key][
                                            :,
                                            tn,
                                            sq * 1024 + half * 512 : sq * 1024 + half * 512 + 512,
                                        ],
                                        cps[:],
                                        biases[key][:, tn : tn + 1],
                                    )

                            return proj_tick

                        proj_fns[hp] = make_proj_tick(chains, chain_ps, chain_act)

                    cols = [
                        (hp, qb, kt)
                        for hp in range(HL // 2)
                        for qb in range(4)
                        for kt in range(NKT)
                    ]

                    def emit_pv(hp, qb, kt, ex):
                        pvs = pv_store[(hp, qb)]
                        for sub in range(2):
                            nc.tensor.matmul(
                                pvs[sub][0 : DV + 1, :],
                                Vp[:, kt, hp * 2 + sub, :],
                                ex[:, sub, :],
                                start=(kt == 0),
                                stop=(kt == NKT - 1),
                            )

                    def emit_norm(hp, qb):
                        t = hp
                        qsl = slice(qb * 512, (qb + 1) * 512)
                        pvs = pv_store.pop((hp, qb))
                        for sub in range(2):
                            psl = slice(sub * 64, sub * 64 + 64)
                            pvp = pvs[sub]
                            rec = rec_pool.tile([1, 512], F32, tag="r")
                            recb = rec_pool.tile([64, 512], F32, tag="rb")
                            dcp = rec_pool.tile([1, 512], F32, tag="d")
                            # custom-DVE ucode mishandles base_partition=64
                            # PSUM reads; stage through partition 0
                            nc.vector.tensor_copy(dcp[:], pvp[DV : DV + 1, :])
                            nc.vector.reciprocal_approx_fast(rec[:], dcp[:])
                            nc.gpsimd.partition_broadcast(recb[:], rec[:])
                            nc.vector.tensor_mul(
                                AT[psl, t, qsl], pvp[0:DV, :], recb[:]
                            )

                    prev = None  # (hp, qb, kt, ex)
                    for hp, qb, kt in cols:
                        t = hp
                        qsl = slice(qb * 512, (qb + 1) * 512)
                        if (hp, qb) not in pv_store:
                            pv_store[(hp, qb)] = [
                                ps_pv.tile([128, 512], F32, tag="pv", name=f"pv{i}")
                                for i in range(2)
                            ]
                        scp = ps_sc.tile([128, 2, 512], F32, tag="sc")
                        for sub in range(2):
                            psl = slice(sub * 64, sub * 64 + 64)
                            nc.tensor.matmul(
                                scp[:, sub, :],
                                KT[psl, t, kt * 128 : (kt + 1) * 128],
                                QT[psl, t, qsl],
                                start=True,
                                stop=True,
                            )
                        ex = exp_pool.tile([128, 2, 512], BF16, tag="e")
                        nc.scalar.activation(ex[:], scp[:], AF.Exp, scale=SCALE)
                        if DEBUG_DUMP and hp == 0 and qb == 0 and kt == 0:
                            nc.sync.dma_start(ex_dbg[:], ex[:, 0, :])
                        if prev is not None:
                            phps, pqb, pkt, pex = prev
                            emit_pv(phps, pqb, pkt, pex)
                            if pkt == NKT - 1:
                                emit_norm(phps, pqb)
                        proj_fns[hp](qb * NKT + kt)
                        prev = (hp, qb, kt, ex)
                    phps, pqb, pkt, pex = prev
                    emit_pv(phps, pqb, pkt, pex)
                    emit_norm(phps, pqb)
                    if DEBUG_DUMP:
                        pass

                if DEBUG_DUMP:
                    nc.sync.dma_start(at_dbg[:], AT[:])
                    nc.sync.dma_start(qt_dbg[:], QT[:])
                    nc.sync.dma_start(kt_dbg[:], KT[:])

                # ---- Stage 3: output projection ----
                with (
                    tc.tile_pool(name="ostage", bufs=3) as ostage,
                    tc.tile_pool(name="ps_o", bufs=2, space="PSUM") as ps_o,
                ):
                    for c in range(NT):
                        nc.sync.dma_start(
                            wo_sb[:, c, :], wo[c * 128 : (c + 1) * 128, :]
                        )
                    for m in range(NDM):
                        for sbk in range(NQB):
                            ps = ps_o.tile([128, 1024], F32, tag="po")
                            for half in range(2):
                                hs = slice(
                                    sbk * 1024 + half * 512,
                                    sbk * 1024 + half * 512 + 512,
                                )
                                for cc in range(NT):
                                    nc.tensor.matmul(
                                        ps[:, half * 512 : half * 512 + 512],
                                        wo_sb[:, cc, m * 128 : (m + 1) * 128],
                                        AT[:, cc, hs],
                                        start=(cc == 0),
                                        stop=(cc == NT - 1),
                                    )
                            ot = ostage.tile([128, 1024], BF16, tag="o")
                            nc.scalar.copy(ot[:], ps[:])
                            nc.sync.dma_start(
                                outT[
                                    m * 128 : (m + 1) * 128,
                                    sbk * 1024 : (sbk + 1) * 1024,
                                ],
                                ot[:],
                            )
            wgt_pool.release()
            acts_pool.release()

    nc.compile()
    return nc


def get_nc():
    global _CACHED_NC
    if _CACHED_NC is None:
        _CACHED_NC = _build()
    return _CACHED_NC


def _bf(x):
    return np.ascontiguousarray(np.asarray(x, np.float32)).astype(NP_BF16)


def make_in_maps(queries, keys, values, Wq, bq, Wk, bk, Wv, bv, Wo, bo):
    queries = np.asarray(queries, np.float32)
    keys = np.asarray(keys, np.float32)
    values = np.asarray(values, np.float32)
    Wq = np.asarray(Wq, np.float32)
    Wk = np.asarray(Wk, np.float32)
    Wv = np.asarray(Wv, np.float32)
    Wo = np.asarray(Wo, np.float32)
    bq = np.asarray(bq, np.float32)
    bk = np.asarray(bk, np.float32)
    bv = np.asarray(bv, np.float32)
    ones = np.ones((1, 128), NP_BF16)
    in_maps = []
    for core in range(8):
        b, hg = divmod(core, 2)
        sl = slice(hg * NB, (hg + 1) * NB)
        in_maps.append(
            {
                "qT": _bf(queries[b].T),
                "kT": _bf(keys[b].T),
                "vT": _bf(values[b].T),
                "wq": _bf(Wq[:, sl]),
                "wk": _bf(Wk[:, sl]),
                "wv": _bf(Wv[:, sl]),
                "wo": _bf(Wo[sl, :]),
                "bq": np.ascontiguousarray(bq[sl]),
                "bk": np.ascontiguousarray(bk[sl]),
                "bv": _bf(bv[sl]),
                "ones": ones,
            }
        )
    return in_maps


def assemble(results, bo):
    bo = np.asarray(bo, np.float32)
    out = np.empty((B, S, DM), np.float32)
    for b in range(B):
        acc = np.asarray(results[2 * b]["outT"], np.float32) + np.asarray(
            results[2 * b + 1]["outT"], np.float32
        )
        out[b] = acc.T + bo
    return out


def run(trace=False, **inputs):
    if trace:
        # NTFF profiling shim: this image's antenv lacks axon_hooks.
        import types

        try:
            from antenv import axon_hooks  # noqa: F401
        except ImportError:
            from trn_agent_boot.trn_boot import _ntff_profile_via_ctypes

            mod = types.ModuleType("antenv.axon_hooks")
            _hook = _ntff_profile_via_ctypes("/opt/axon/libaxon_pjrt.so")
            mod.get_axon_ntff_profile_hook = lambda: _hook
            sys.modules["antenv.axon_hooks"] = mod
    nc = get_nc()
    bo = inputs["bo"]
    in_maps = make_in_maps(**inputs)
    res = run_bass_kernel_spmd(nc, in_maps, list(range(8)), trace=trace)
    return assemble(res.results, bo), res


def kernel(**inputs):
    out, _ = run(trace=False, **inputs)
    return out
